# revision 29
# baseline (speedup 1.0000x reference)
"""Trainium2 Bass kernel: 7x7 valid cross-correlation + bias on a 4096x4096 f32 image.

Formulation: banded matmul on the TensorEngine.
  out[r, c] = sum_{di,dj} w[di,dj] * x[r+di, c+dj]
For an output row-strip of M=122 rows starting at r0, using K=128 input rows:
  out[r0+m, c] = sum_k A_dj[k, m] * x[r0+k, c+dj]   summed over dj=0..6
where A_dj[k, m] = w[k-m, dj] for 0 <= k-m < 7 (a banded [128, 122] matrix,
precomputed on host from the 49 kernel weights). The 7 dj-terms accumulate
into one PSUM bank via shifted column slices of the same SBUF rhs tile.

Matmuls run in bf16 (216ns per N=512 matmul warm — the PE roofline for this
formulation); output written back as bf16, upcast on host (rel-err ~4e-3 vs
the 2e-2 gate).

Schedule (v12, trace-driven — see git/-style history in comments):
  - exec_time spans first USER instruction -> end of NEFF epilogue (the
    ~6.3us full-range semaphore sweep at the end is fixed cost; the ~6us
    framework preamble is excluded).
  - HAM warmup: ~7 dummy matmuls (cold 427ns each) on a memset scratch
    during the input-DMA spin-up so real matmuls run at 2.4GHz.
  - Inputs on the Sync HWDGE queue in-order: bands first, then strip chunks
    sized so arrival stays ahead of the warm PE.
  - Stores are SWDGE (gpsimd) only, one per PSUM group, and span ALL 128
    partitions of a contiguous per-group dram tensor: non-128-partition
    SWDGE stores take a slow descriptor path (~100GB/s + a storm of tiny
    ring packets); 128-partition contiguous stores run ~350GB/s clean.
    Rows 122-127 of each strip are garbage the host discards.
  - Vector offload: strips 4-5 are computed on the (otherwise underused)
    Vector engine as 49-tap FMA chains (scalar_tensor_tensor, f32 accum),
    interleaved with the PE pipeline; all PSUM drains move to Scalar
    (activation Identity + bias). This shaves 2 strips (~3us) off the PE
    critical path.

Sharding: output columns split across the 8 cores (512 cols/core); each
core processes all 4090 output rows. Kernel + bias replicated.
"""

import numpy as np

H, W = 4096, 4096
KH, KW = 7, 7
OH, OW = H - KH + 1, W - KW + 1  # 4090, 4090
N_CORES = 8
CW = 512               # output columns per core
IW = CW + KW - 1       # input columns per core (518)
STRIP = 122            # output rows per strip (K = STRIP + KH - 1 = 128)
MB = 128               # stationary block columns (M padded 122 -> 128)
N_STRIPS = (OH + STRIP - 1) // STRIP  # 34 (last strip M=64, K=70)

OFFLOAD = [4, 5]       # strips computed on Vector instead of the PE
PE_GROUPS = (
    [[0, 1], [2, 3]]
    + [[s, s + 1] for s in range(6, 32, 2)]
    + [[32], [33]]
)
# input DMA chunks (PE strips only; offloaded strips ship as xoff tensors)
IN_CHUNK_STRIPS = [
    [0, 1],
    [2, 3],
    [6, 7, 8, 9],
    [10, 11, 12, 13, 14, 15],
    [16, 17, 18, 19, 20, 21, 22],
    [23, 24, 25, 26, 27, 28, 29],
    [30, 31, 32, 33],
]
N_WARM = 7                                       # dummy matmuls for HAM warmup
TAPS_PER_GROUP = 8     # Vector FMA taps emitted between PE groups

assert sorted([s for g in PE_GROUPS for s in g] + OFFLOAD) == list(range(N_STRIPS))
assert sorted(s for ch in IN_CHUNK_STRIPS for s in ch) == sorted(
    s for g in PE_GROUPS for s in g
)

_cache = {}


def _build_nc():
    import concourse.bacc as bacc
    import concourse.mybir as mybir
    from concourse.tile import TileContext

    f32 = mybir.dt.float32
    bf16 = mybir.dt.bfloat16

    nc = bacc.Bacc("TRN2", target_bir_lowering=False, debug=False)
    xs = nc.dram_tensor("xs", [128, N_STRIPS * IW], bf16, kind="ExternalInput")
    # Per offloaded strip: 7 di-shifted copies of its input rows, packed
    # [k, di*IW + c] = x[s*STRIP + di + k, c0 + c] — DVE ops cannot read at
    # a partition offset, so the di shifts are materialized host-side.
    xoffs = {
        s: nc.dram_tensor(f"xoff{s}", [128, KH * IW], bf16, kind="ExternalInput")
        for s in OFFLOAD
    }
    bands = nc.dram_tensor("bands", [128, KW * MB], bf16, kind="ExternalInput")
    biasv = nc.dram_tensor("biasv", [128, 1], f32, kind="ExternalInput")
    wvec = nc.dram_tensor("wvec", [128, KH * KW], f32, kind="ExternalInput")
    outs = {}  # key: tuple of strips -> dram tensor [128, n*CW]
    for g in PE_GROUPS:
        outs[tuple(g)] = nc.dram_tensor(
            f"out_{g[0]}", [128, len(g) * CW], bf16, kind="ExternalOutput"
        )
    for s in OFFLOAD:
        outs[(s,)] = nc.dram_tensor(
            f"out_{s}", [128, CW], bf16, kind="ExternalOutput"
        )

    with TileContext(nc) as tc:
        with (
            tc.tile_pool(name="const", bufs=1) as cpool,
            tc.tile_pool(name="rhs", bufs=6) as rpool,
            tc.tile_pool(name="obuf", bufs=8) as opool,
            tc.tile_pool(name="acc", bufs=len(OFFLOAD)) as apool,
            tc.tile_pool(name="psum", bufs=8, space="PSUM") as ppool,
        ):
            # Warmup scratch on GpSimd (earliest-free engine) so the PE's
            # warmup burst starts as soon as possible after the preamble.
            warm_t = cpool.tile([128, 640], bf16)
            nc.gpsimd.memset(warm_t[:, :], 0.0)
            bias1_t = cpool.tile([128, 1], f32)
            nc.scalar.dma_start(out=bias1_t[:, :], in_=biasv[:, :])
            wv_t = cpool.tile([128, KH * KW], f32)
            nc.scalar.dma_start(out=wv_t[:, :], in_=wvec[:, :])

            # Input loads on the Sync HWDGE queue, in-order, bands first.
            band_t = cpool.tile([128, KW * MB], bf16)
            nc.sync.dma_start(out=band_t[:, :], in_=bands[:, :])
            strip_tile = {}
            xoff_t = {}
            max_chunk = max(len(ch) for ch in IN_CHUNK_STRIPS)
            for ci, ch in enumerate(IN_CHUNK_STRIPS):
                n = len(ch)
                s0 = ch[0]
                assert ch == list(range(s0, s0 + n))
                xt = rpool.tile([128, max_chunk * IW], bf16, tag="rhs")
                nc.sync.dma_start(
                    out=xt[:, : n * IW], in_=xs[:, s0 * IW : (s0 + n) * IW]
                )
                for j, s in enumerate(ch):
                    strip_tile[s] = (xt, j * IW)
                if ci == 1:
                    # shifted copies for the offloaded strips ride between
                    # the early chunks: needed from ~17us, land by ~15us
                    for s in OFFLOAD:
                        xo = cpool.tile([128, KH * IW], bf16, name=f"xo{s}")
                        nc.sync.dma_start(out=xo[:, :], in_=xoffs[s][:, :])
                        xoff_t[s] = xo

            # HAM warmup: ~3.4us of dummy matmuls (cold: 427ns each) so the
            # PE clock is at 2.4GHz when real matmuls start.
            warm_ps = ppool.tile([128, CW], f32, name="ps", tag="ps")
            for _ in range(N_WARM):
                nc.tensor.matmul(
                    warm_ps[:, :],
                    warm_t[:, :128],
                    warm_t[:, 128:640],
                    start=True,
                    stop=True,
                )

            # Vector-offload machinery: per offloaded strip a f32 accumulator
            # and a generator yielding its 49 FMA taps one at a time.
            acc_t = {
                s: apool.tile([128, CW], f32, name=f"acc{s}", tag="acc")
                for s in OFFLOAD
            }

            def conv_taps(s):
                xo = xoff_t[s]
                acc = acc_t[s]
                t = 0
                for di in range(KH):
                    for dj in range(KW):
                        xw = xo[:STRIP, di * IW + dj : di * IW + dj + CW]
                        if t == 0:
                            nc.vector.tensor_scalar_mul(
                                acc[:STRIP, :], xw, wv_t[:STRIP, t : t + 1]
                            )
                        else:
                            nc.vector.scalar_tensor_tensor(
                                acc[:STRIP, :],
                                xw,
                                wv_t[:STRIP, t : t + 1],
                                acc[:STRIP, :],
                                mybir.AluOpType.mult,
                                mybir.AluOpType.add,
                            )
                        t += 1
                        yield
                # finalize: add bias, convert to bf16, store
                ot = opool.tile([128, CW], bf16, tag="ot")
                nc.vector.tensor_scalar_add(
                    ot[:STRIP, :], acc[:STRIP, :], bias1_t[:STRIP, :1]
                )
                nc.gpsimd.dma_start(out=outs[(s,)][:, :], in_=ot[:, :])
                yield

            tap_gen = None
            tap_queue = list(OFFLOAD)

            def emit_taps(k):
                nonlocal tap_gen
                for _ in range(k):
                    if tap_gen is None:
                        if not tap_queue:
                            return
                        tap_gen = conv_taps(tap_queue.pop(0))
                    try:
                        next(tap_gen)
                    except StopIteration:
                        tap_gen = None

            for gi, strips in enumerate(PE_GROUPS):
                n = len(strips)
                dims = []
                for s in strips:
                    r0 = s * STRIP
                    dims.append((r0, min(STRIP, OH - r0), min(128, H - r0)))
                ps_ts = [
                    ppool.tile([128, CW], f32, name="ps", tag="ps") for _ in strips
                ]
                for dj in range(KW):
                    lhsT = band_t[:, dj * MB : dj * MB + MB]
                    for j, (r0, M, K) in enumerate(dims):
                        sxt, soff = strip_tile[strips[j]]
                        nc.tensor.matmul(
                            ps_ts[j][:, :],
                            lhsT[:K, :],
                            sxt[:K, soff + dj : soff + dj + CW],
                            start=(dj == 0),
                            stop=(dj == KW - 1),
                        )
                # All PSUM drains on Scalar (single producer per store);
                # Vector is reserved for the offloaded-strip FMA chains.
                ot = opool.tile([128, 2 * CW], bf16, tag="ot")
                for j, (r0, M, K) in enumerate(dims):
                    nc.scalar.activation(
                        ot[:M, j * CW : (j + 1) * CW],
                        ps_ts[j][:M, :],
                        mybir.ActivationFunctionType.Identity,
                        bias=bias1_t[:M, :1],
                    )
                nc.gpsimd.dma_start(
                    out=outs[tuple(strips)][:, :],
                    in_=ot[:, : n * CW],
                )
                if gi >= 1:
                    emit_taps(TAPS_PER_GROUP)
            emit_taps(2 * (KH * KW + 1))  # flush any remaining taps

    nc.finalize()
    return nc


def _get_nc():
    if "nc" not in _cache:
        _cache["nc"] = _build_nc()
    return _cache["nc"]


def _build_bands(weight: np.ndarray) -> np.ndarray:
    """bands[k, dj*MB + m] = weight[k - m, dj] for 0 <= k-m < KH, m < STRIP."""
    w = np.asarray(weight, np.float32)
    bands = np.zeros((128, KW * MB), np.float32)
    m = np.arange(STRIP)
    for dj in range(KW):
        for di in range(KH):
            bands[m + di, dj * MB + m] = w[di, dj]
    return bands


def _prepare_in_maps(x, weight, bias):
    import ml_dtypes

    bf16 = ml_dtypes.bfloat16
    xb = np.ascontiguousarray(x, np.float32).astype(bf16)
    bands = _build_bands(weight).astype(bf16)
    bias_tile = np.full((128, 1), np.float32(np.asarray(bias).reshape(-1)[0]))
    w = np.asarray(weight, np.float32)
    wvec = np.broadcast_to(w.reshape(1, KH * KW), (128, KH * KW)).copy()

    # xs_packed[k, s, c] = x[122*s + k, c0 + c], zero beyond image edges.
    k_idx = np.arange(128)[:, None]
    s_idx = np.arange(N_STRIPS)[None, :]
    rows = k_idx + STRIP * s_idx  # [128, N_STRIPS]
    row_ok = rows < H
    rows_c = np.minimum(rows, H - 1)

    k128 = np.arange(128)[:, None]
    di7 = np.arange(KH)[None, :]

    in_maps = []
    for c in range(N_CORES):
        c0 = c * CW
        avail = min(IW, W - c0)
        xsl = np.zeros((H, IW), bf16)
        xsl[:, :avail] = xb[:, c0 : c0 + avail]
        xs = xsl[rows_c, :]  # [128, N_STRIPS, IW]
        xs[~row_ok] = 0
        xs = np.ascontiguousarray(xs.reshape(128, N_STRIPS * IW))
        m = {"xs": xs, "bands": bands, "biasv": bias_tile, "wvec": wvec}
        for s in OFFLOAD:
            r = s * STRIP + di7 + k128  # [128, KH]
            rc = np.minimum(r, H - 1)
            xo = xsl[rc, :]  # [128, KH, IW]
            xo[r >= H] = 0
            m[f"xoff{s}"] = np.ascontiguousarray(xo.reshape(128, KH * IW))
        in_maps.append(m)
    return in_maps


def _gather_out(per_core_outs) -> np.ndarray:
    groups = [list(g) for g in PE_GROUPS] + [[s] for s in OFFLOAD]
    out = np.empty((OH, OW), np.float32)
    for c in range(N_CORES):
        c0 = c * CW
        take = min(CW, OW - c0)
        full = np.empty((N_STRIPS * STRIP, CW), np.float32)
        for g in groups:
            og = per_core_outs[c][f"out_{g[0]}"].astype(np.float32)
            og = og.reshape(128, len(g), CW)[:STRIP].transpose(1, 0, 2)
            for j, s in enumerate(g):
                full[s * STRIP : (s + 1) * STRIP] = og[j]
        out[:, c0 : c0 + take] = full[:OH, :take]
    return out


def kernel(x: np.ndarray, weight: np.ndarray, bias: np.ndarray) -> np.ndarray:
    from concourse import bass_utils

    nc = _get_nc()
    in_maps = _prepare_in_maps(x, weight, bias)
    res = bass_utils.run_bass_kernel_spmd(nc, in_maps, list(range(N_CORES)))
    _cache["last_results"] = res
    return _gather_out(res.results)


# revision 30
# speedup vs baseline: 1.3189x; 1.3189x over previous
"""Trainium2 Bass kernel: 7x7 valid cross-correlation + bias on a 4096x4096 f32 image.

Formulation: banded matmul on the TensorEngine.
  out[r, c] = sum_{di,dj} w[di,dj] * x[r+di, c+dj]
For an output row-strip of M=122 rows starting at r0, using K=128 input rows:
  out[r0+m, c] = sum_k A_dj[k, m] * x[r0+k, c+dj]   summed over dj=0..6
where A_dj[k, m] = w[k-m, dj] for 0 <= k-m < 7 (a banded [128, 122] matrix,
precomputed on host from the 49 kernel weights). The 7 dj-terms accumulate
into one PSUM bank via shifted column slices of the same SBUF rhs tile.

Matmuls run in bf16 (216ns per N=512 matmul warm — the PE roofline for this
formulation; 34 strips x 7 dj ~ 51.5us/core is the compute floor); output
written back as bf16, upcast on host (rel-err ~4e-3 vs the 2e-2 gate).

Schedule (trace-driven):
  - exec_time spans first USER instruction -> end of NEFF epilogue (the
    ~6.3us full-range semaphore sweep at the end is fixed framework cost;
    the ~6us engine preamble is excluded from the measurement).
  - HAM warmup: 7 dummy matmuls (cold 427ns each) on a memset scratch while
    the input DMAs spin up, so all real matmuls issue at 2.4GHz.
  - Inputs on the Sync HWDGE queue in-order: bands first, then strip chunks
    [2,3,5,6,7,7,4] — delivery (~290GB/s) stays ahead of a warm PE (~88GB/s)
    with no mid-stream stalls (stalls also reset the HAM warmup window).
  - PSUM groups of 2 strips ([2]*16+[1,1]); dj is the outer loop within a
    group so matmuls sharing a stationary band run back-to-back; 8-bank
    PSUM pool gives a reuse distance of 4 groups.
  - Drains alternate whole-group Vector/Scalar so every store has a single
    producer semaphore (a second producer pushes the wait into the DMA
    ring, where SDMA engines poll it with tiny packets).
  - Stores are SWDGE (gpsimd) only — HWDGE SBUF->HBM receipts take +10us+.
    One store per group into a per-group contiguous dram tensor spanning
    ALL 128 partitions: non-128-partition SWDGE stores take a slow
    descriptor path (~100GB/s + a storm of tiny ring packets); full-128
    contiguous stores run ~350GB/s clean. Rows 122-127 are garbage the
    host discards. Small (2-strip) stores keep the end-of-kernel in-flight
    backlog and the final completion receipt short.

Sharding: output columns split across the 8 cores (512 cols/core); each
core processes all 4090 output rows. Kernel + bias replicated.
"""

import numpy as np

H, W = 4096, 4096
KH, KW = 7, 7
OH, OW = H - KH + 1, W - KW + 1  # 4090, 4090
N_CORES = 8
CW = 512               # output columns per core
IW = CW + KW - 1       # input columns per core (518)
STRIP = 122            # output rows per strip (K = STRIP + KH - 1 = 128)
MB = 128               # stationary block columns (M padded 122 -> 128)
N_STRIPS = (OH + STRIP - 1) // STRIP  # 34 (last strip M=64, K=70)

GROUPS = [2] * 16 + [1, 1]                      # strips per PSUM group
IN_CHUNKS = [2, 3, 5, 6, 7, 7, 4]               # strips per input DMA
N_WARM = 7                                       # dummy matmuls for HAM warmup

assert sum(GROUPS) == N_STRIPS and sum(IN_CHUNKS) == N_STRIPS

_cache = {}


def _build_nc():
    import concourse.bacc as bacc
    import concourse.mybir as mybir
    from concourse.tile import TileContext

    f32 = mybir.dt.float32
    bf16 = mybir.dt.bfloat16

    nc = bacc.Bacc("TRN2", target_bir_lowering=False, debug=False)
    xs = nc.dram_tensor("xs", [128, N_STRIPS * IW], bf16, kind="ExternalInput")
    bands = nc.dram_tensor("bands", [128, KW * MB], bf16, kind="ExternalInput")
    biasv = nc.dram_tensor("biasv", [128, 1], f32, kind="ExternalInput")
    # One output tensor per group: out_g[m, j*CW+c] = out_full[(s0+j)*STRIP+m, c]
    outs = [
        nc.dram_tensor(f"out{gi}", [128, n * CW], bf16, kind="ExternalOutput")
        for gi, n in enumerate(GROUPS)
    ]

    with TileContext(nc) as tc:
        with (
            tc.tile_pool(name="const", bufs=1) as cpool,
            tc.tile_pool(name="rhs", bufs=6) as rpool,
            tc.tile_pool(name="obuf", bufs=8) as opool,
            tc.tile_pool(name="psum", bufs=8, space="PSUM") as ppool,
        ):
            # Warmup scratch on GpSimd (earliest-free engine) so the PE's
            # warmup burst starts as soon as possible after the preamble.
            warm_t = cpool.tile([128, 640], bf16)
            nc.gpsimd.memset(warm_t[:, :], 0.0)
            bias1_t = cpool.tile([128, 1], f32)
            nc.scalar.dma_start(out=bias1_t[:, :], in_=biasv[:, :])

            # Input loads on the Sync HWDGE queue, in-order, bands first.
            band_t = cpool.tile([128, KW * MB], bf16)
            nc.sync.dma_start(out=band_t[:, :], in_=bands[:, :])
            strip_tile = {}
            s0 = 0
            for n in IN_CHUNKS:
                xt = rpool.tile([128, max(IN_CHUNKS) * IW], bf16, tag="rhs")
                nc.sync.dma_start(
                    out=xt[:, : n * IW], in_=xs[:, s0 * IW : (s0 + n) * IW]
                )
                for j in range(n):
                    strip_tile[s0 + j] = (xt, j * IW)
                s0 += n

            # HAM warmup: ~3.4us of dummy matmuls (cold: 427ns each) so the
            # PE clock is at 2.4GHz when real matmuls start.
            warm_ps = ppool.tile([128, CW], f32, name="ps", tag="ps")
            for _ in range(N_WARM):
                nc.tensor.matmul(
                    warm_ps[:, :],
                    warm_t[:, :128],
                    warm_t[:, 128:640],
                    start=True,
                    stop=True,
                )
            # broadcast bias to [128, CW] on-chip for the Vector drains
            bias_t = cpool.tile([128, CW], f32)
            nc.vector.tensor_scalar_add(
                bias_t[:, :], warm_t[:, :CW], bias1_t[:, :1]
            )

            strips_done = 0
            for gi, n in enumerate(GROUPS):
                s0 = strips_done
                strips = list(range(s0, s0 + n))
                strips_done += n
                dims = []
                for s in strips:
                    r0 = s * STRIP
                    dims.append((r0, min(STRIP, OH - r0), min(128, H - r0)))
                ps_ts = [
                    ppool.tile([128, CW], f32, name="ps", tag="ps") for _ in strips
                ]
                for dj in range(KW):
                    lhsT = band_t[:, dj * MB : dj * MB + MB]
                    for j, (r0, M, K) in enumerate(dims):
                        sxt, soff = strip_tile[strips[j]]
                        nc.tensor.matmul(
                            ps_ts[j][:, :],
                            lhsT[:K, :],
                            sxt[:K, soff + dj : soff + dj + CW],
                            start=(dj == 0),
                            stop=(dj == KW - 1),
                        )
                ot = opool.tile([128, max(GROUPS) * CW], bf16, tag="ot")
                for j, (r0, M, K) in enumerate(dims):
                    if gi % 2 == 0:
                        nc.vector.tensor_tensor(
                            ot[:M, j * CW : (j + 1) * CW],
                            ps_ts[j][:M, :],
                            bias_t[:M, :],
                            mybir.AluOpType.add,
                        )
                    else:
                        nc.scalar.activation(
                            ot[:M, j * CW : (j + 1) * CW],
                            ps_ts[j][:M, :],
                            mybir.ActivationFunctionType.Identity,
                            bias=bias1_t[:M, :1],
                        )
                nc.gpsimd.dma_start(
                    out=outs[gi][:, :],
                    in_=ot[:, : n * CW],
                )

    nc.finalize()
    return nc


def _get_nc():
    if "nc" not in _cache:
        _cache["nc"] = _build_nc()
    return _cache["nc"]


def _build_bands(weight: np.ndarray) -> np.ndarray:
    """bands[k, dj*MB + m] = weight[k - m, dj] for 0 <= k-m < KH, m < STRIP."""
    w = np.asarray(weight, np.float32)
    bands = np.zeros((128, KW * MB), np.float32)
    m = np.arange(STRIP)
    for dj in range(KW):
        for di in range(KH):
            bands[m + di, dj * MB + m] = w[di, dj]
    return bands


def _prepare_in_maps(x, weight, bias):
    import ml_dtypes

    bf16 = ml_dtypes.bfloat16
    xb = np.ascontiguousarray(x, np.float32).astype(bf16)
    bands = _build_bands(weight).astype(bf16)
    bias_tile = np.full((128, 1), np.float32(np.asarray(bias).reshape(-1)[0]))

    # xs_packed[k, s, c] = x[122*s + k, c0 + c], zero beyond image edges.
    k_idx = np.arange(128)[:, None]
    s_idx = np.arange(N_STRIPS)[None, :]
    rows = k_idx + STRIP * s_idx  # [128, N_STRIPS]
    row_ok = rows < H
    rows_c = np.minimum(rows, H - 1)

    in_maps = []
    for c in range(N_CORES):
        c0 = c * CW
        avail = min(IW, W - c0)
        xsl = np.zeros((H, IW), bf16)
        xsl[:, :avail] = xb[:, c0 : c0 + avail]
        xs = xsl[rows_c, :]  # [128, N_STRIPS, IW]
        xs[~row_ok] = 0
        xs = np.ascontiguousarray(xs.reshape(128, N_STRIPS * IW))
        in_maps.append({"xs": xs, "bands": bands, "biasv": bias_tile})
    return in_maps


def _gather_out(per_core_outs) -> np.ndarray:
    out = np.empty((OH, OW), np.float32)
    for c in range(N_CORES):
        c0 = c * CW
        take = min(CW, OW - c0)
        full = np.empty((N_STRIPS * STRIP, CW), np.float32)
        s0 = 0
        for gi, n in enumerate(GROUPS):
            og = per_core_outs[c][f"out{gi}"].astype(np.float32)
            og = og.reshape(128, n, CW)[:STRIP].transpose(1, 0, 2)
            full[s0 * STRIP : (s0 + n) * STRIP] = og.reshape(n * STRIP, CW)
            s0 += n
        out[:, c0 : c0 + take] = full[:OH, :take]
    return out


def kernel(x: np.ndarray, weight: np.ndarray, bias: np.ndarray) -> np.ndarray:
    from concourse import bass_utils

    nc = _get_nc()
    in_maps = _prepare_in_maps(x, weight, bias)
    res = bass_utils.run_bass_kernel_spmd(nc, in_maps, list(range(N_CORES)))
    _cache["last_results"] = res
    return _gather_out(res.results)
